# revision 2
# baseline (speedup 1.0000x reference)
"""ConfSMoE Trainium2 kernel — capacity-based expert dispatch across 8 cores.

Strategy
-------------
B,S,D,E,H = 8,512,512,8,2048; top-2-of-8 confidence-weighted MoE.
Dense expert compute (baseline) does 4x the needed FLOPs. Instead:

- Host computes the routing DECISION only (which tokens go to which
  expert) and gathers, per core e, the raw x rows of the tokens whose
  top-2 (plus near-ties within 7e-3) includes expert e, grouped by
  source batch with a fixed per-(expert,batch) capacity CAP=192
  (observed max cell 148).  Pads point at a real row with
  position=4096 (never selected by the one-hot).  All parameter-style
  inputs arrive pre-laid-out [128, n] so every DMA is contiguous.
- Core e (all math on device): LayerNorm its 1536 slots (fp32 stats),
  PE-transpose, bf16 router logits, softmax + top-2 mask (Max8/is_ge)
  + renormalized confidence of its own expert; FFN in bf16 over
  512-slot segments; gate-scale on vector; weights arrive pre-cast
  bf16 on the gpsimd DMA queue while activations use the sync queue.
- Positioning: one-hot slot->position matrices (device-built from the
  host slot-position vector vs an iota row) matmul the slot-major
  outputs into token-position order; partial rows are written in a
  core-interleaved PAIRED layout so one bf16 ReduceScatter serves two
  batches (4 collectives total, fired as soon as the second batch of
  the pair is positioned, overlapping the remaining FFN segments).
- Each core ends with 64 rows of each batch (two batches stacked per
  RS): residual + final LayerNorm on [128, D] pair tiles.

Engine budget: PE does transposes/router/FFN/positioning; scalar only
sqrt/exp/gelu in contiguous blocks (minimal activation-table reloads);
vector does LN/router vector work and gate scales; gpsimd does weight
DMAs, partial bf16 casts and the collectives.

Device routing is self-consistent across cores; host dispatch only
needs to be a superset of the device's top-2, guaranteed by also
dispatching experts within 7e-3 of the 2nd-highest logit (bf16 router
perturbs logits by <5e-3 on this data).
"""

import numpy as np
import ml_dtypes

import concourse.bass as bass
import concourse.mybir as mybir
import concourse.tile as tile
from concourse import bacc
from concourse.bass_utils import run_bass_kernel_spmd
from concourse.masks import make_identity

B, S, D, E, H = 8, 512, 512, 8, 2048
N_CORES = 8
T = B * S              # 4096 tokens
EPS = 1e-5
P = 128                # partitions
KD = D // P            # 4  D-tiles
KH = H // P            # 16 H-tiles
CAP = 192              # slots per (expert, batch); CAP*b % 128 must be in
                       # {0, 64}: matmul base partition must be 0/32/64
NSLOT = B * CAP        # 1536 slots per core
NTILE = NSLOT // P     # 12 slot tiles
WT = 4                 # slot tiles per wave / FFN segment
NW = NTILE // WT       # 3 waves
PF = S // N_CORES      # 64 output rows per (core, batch)
NPAIR = B // 2         # 4 paired ReduceScatters
PAD_POS = float(T)     # one-hot never matches

FP32 = mybir.dt.float32
BF16 = mybir.dt.bfloat16
AF = mybir.ActivationFunctionType
ALU = mybir.AluOpType
AX = mybir.AxisListType


def _bc(dram_param, p, n):
    """[n]-shaped DRAM tensor broadcast to [p, n] via a step-0 partition dim."""
    a = dram_param.ap()
    return bass.AP(tensor=a.tensor, offset=a.offset, ap=[[0, p]] + list(a.ap))


def _cover(b):
    """Slot-tile chunks (tile, row0, nrows) covering batch b's CAP slots."""
    lo, hi = CAP * b, CAP * (b + 1)
    chunks = []
    while lo < hi:
        t, r = lo // P, lo % P
        n = min(P - r, hi - lo)
        chunks.append((t, r, n))
        lo += n
    return chunks


# batches whose last covering slot tile is g
_POS_AFTER = {}
for _b in range(B):
    _g = (CAP * (_b + 1) + P - 1) // P - 1
    _POS_AFTER.setdefault(_g, []).append(_b)


def build(ln_g1=False, ln_b0=False, out_g1=False, out_b0=False, b2_0=False):
    nc = bacc.Bacc("TRN2", target_bir_lowering=False, debug=False,
                   num_devices=N_CORES)

    # ---------------- I/O (host pre-laid-out, contiguous DMAs) ----------------
    xg_d = nc.declare_dram_parameter("xg", [NSLOT, D], FP32, isOutput=False)
    pos_d = nc.declare_dram_parameter("pos", [P, NTILE], FP32, isOutput=False)
    iota_d = nc.declare_dram_parameter("iota", [P, S], FP32, isOutput=False)
    x_res = nc.declare_dram_parameter("x_res", [NPAIR, 2 * PF, D], FP32,
                                      isOutput=False)
    wg_d = nc.declare_dram_parameter("Wgt", [P, KD * E], FP32, isOutput=False)
    W1_d = nc.declare_dram_parameter("W1e", [D, H], BF16, isOutput=False)
    b1_d = nc.declare_dram_parameter("b1e", [P, KH], FP32, isOutput=False)
    W2_d = nc.declare_dram_parameter("W2e", [H, D], BF16, isOutput=False)
    b2_d = nc.declare_dram_parameter("b2e", [D], FP32, isOutput=False)
    ln_g_d = nc.declare_dram_parameter("ln_g", [D], FP32, isOutput=False)
    ln_b_d = nc.declare_dram_parameter("ln_b", [D], FP32, isOutput=False)
    out_g_d = nc.declare_dram_parameter("out_g", [D], FP32, isOutput=False)
    out_b_d = nc.declare_dram_parameter("out_b", [D], FP32, isOutput=False)
    sel_d = nc.declare_dram_parameter("sel", [P, E], FP32, isOutput=False)
    out_d = nc.declare_dram_parameter("out", [NPAIR, 2 * PF, D], FP32,
                                      isOutput=True)

    # ------------- internal DRAM (paired collectives) -------------
    # partial2[p] rows: 128*j + 64*half + r  ->  core j, batch 2p+half, row r
    partial_dram = [nc.dram_tensor(f"partial{p}", [2 * S, D], BF16)
                    for p in range(NPAIR)]
    rs_out_dram = [nc.dram_tensor(f"rs_out{p}", [2 * PF, D], BF16)
                   for p in range(NPAIR)]

    rg = [list(range(N_CORES))]

    with tile.TileContext(nc) as tc:
        with (
            tc.tile_pool(name="params", bufs=1) as ppool,
            tc.tile_pool(name="wts", bufs=1) as wpool,
            tc.tile_pool(name="xst", bufs=8) as xpool,
            tc.tile_pool(name="xnT", bufs=1) as xnpool,
            tc.tile_pool(name="route", bufs=2) as rpool,
            tc.tile_pool(name="hT", bufs=1) as hpool,
            tc.tile_pool(name="ob", bufs=1) as opool,
            tc.tile_pool(name="part", bufs=4) as cpool,
            tc.tile_pool(name="fin", bufs=2) as fpool,
            tc.tile_pool(name="ps_tr", bufs=1, space="PSUM") as ps_tr,
            tc.tile_pool(name="ps_lg", bufs=1, space="PSUM") as ps_lg,
            tc.tile_pool(name="ps_h", bufs=2, space="PSUM") as ps_h,
            tc.tile_pool(name="ps_o", bufs=2, space="PSUM") as ps_o,
            tc.tile_pool(name="ps_p", bufs=2, space="PSUM") as ps_p,
        ):
            # ---------------- constants / params ----------------
            G1 = B1t = OG = OB = None
            if not ln_g1:
                G1 = ppool.tile([P, D], FP32, tag="G1")
                nc.gpsimd.dma_start(out=G1, in_=_bc(ln_g_d, P, D))
            if not ln_b0:
                B1t = ppool.tile([P, D], FP32, tag="B1t")
                nc.gpsimd.dma_start(out=B1t, in_=_bc(ln_b_d, P, D))
            if not out_g1:
                OG = ppool.tile([P, D], FP32, tag="OG")
                nc.gpsimd.dma_start(out=OG, in_=_bc(out_g_d, P, D))
            if not out_b0:
                OB = ppool.tile([P, D], FP32, tag="OB")
                nc.gpsimd.dma_start(out=OB, in_=_bc(out_b_d, P, D))
            if not b2_0:
                B2 = ppool.tile([P, D], FP32, tag="B2")
                nc.gpsimd.dma_start(out=B2, in_=_bc(b2_d, P, D))

            eps_t = ppool.tile([P, 1], FP32, tag="eps")
            nc.vector.memset(eps_t, EPS)

            ident = ppool.tile([P, P], FP32, tag="ident")
            make_identity(nc, ident)

            # first-wave activations ahead of everything on the sync queue
            xts = [None] * NTILE
            for t in range(WT):
                xt = xpool.tile([P, D], FP32, tag="xg_t")
                nc.sync.dma_start(out=xt, in_=xg_d[t * P:(t + 1) * P, :])
                xts[t] = xt

            wg_sb = ppool.tile([P, KD, E], FP32, tag="wg_sb")
            nc.sync.dma_start(out=wg_sb, in_=wg_d.ap())
            b1_sb = ppool.tile([P, KH], FP32, tag="b1_sb")
            nc.sync.dma_start(out=b1_sb, in_=b1_d.ap())
            pos_sb = ppool.tile([P, NTILE], FP32, tag="pos_sb")
            nc.sync.dma_start(out=pos_sb, in_=pos_d.ap())
            iota_sb = ppool.tile([P, S], FP32, tag="iota_sb")
            nc.sync.dma_start(out=iota_sb, in_=iota_d.ap())
            sel_bc = ppool.tile([P, E], FP32, tag="sel_bc")
            nc.sync.dma_start(out=sel_bc, in_=sel_d.ap())

            # weights: pre-cast bf16, on the gpsimd DMA queue
            w1_bf = []
            for k in range(KD):
                wbf = wpool.tile([P, H], BF16, tag=f"w1_bf{k}")
                nc.gpsimd.dma_start(out=wbf, in_=W1_d[k * P:(k + 1) * P, :])
                w1_bf.append(wbf)
            w2_bf = []
            for m in range(KH):
                wbf = wpool.tile([P, D], BF16, tag=f"w2_bf{m}")
                nc.gpsimd.dma_start(out=wbf, in_=W2_d[m * P:(m + 1) * P, :])
                w2_bf.append(wbf)

            xgT_bf = []        # [KD] tiles [P, NSLOT] bf16 (FFN rhs / router lhsT)
            for d in range(KD):
                xd = xnpool.tile([P, NSLOT], BF16, tag=f"xgT{d}")
                xgT_bf.append(xd)

            gts = [None] * NTILE     # [P, 1] fp32: own-expert confidence
            sel_oh = [None] * NTILE  # [P, S] bf16: slot -> position one-hot
            o_bf = [None] * NTILE    # [P, D] bf16: gate-scaled expert outputs
            tmpT = {}                # (t, d) -> [P, P] fp32 (fp32 router lhsT)

            def wave_A(w):
                """LayerNorm + transpose + router for slot tiles of wave w."""
                tiles = range(w * WT, (w + 1) * WT)
                for t in tiles:
                    if xts[t] is None:
                        xt = xpool.tile([P, D], FP32, tag="xg_t")
                        nc.sync.dma_start(out=xt,
                                          in_=xg_d[t * P:(t + 1) * P, :])
                        xts[t] = xt
                for t in tiles:
                    xt = xts[t]
                    stats = rpool.tile([P, 6], FP32, tag="stats")
                    nc.vector.bn_stats(out=stats, in_=xt)
                    mv = rpool.tile([P, 2], FP32, tag="mv")
                    nc.vector.bn_aggr(out=mv, in_=stats)
                    sd = rpool.tile([P, 1], FP32, tag="sd")
                    nc.scalar.activation(out=sd, in_=mv[:, 1:2], func=AF.Sqrt,
                                         bias=eps_t, scale=1.0)
                    rstd = rpool.tile([P, 1], FP32, tag="rstd")
                    nc.vector.reciprocal(out=rstd, in_=sd)
                    nc.vector.tensor_scalar(out=xt, in0=xt, scalar1=mv[:, 0:1],
                                            scalar2=rstd, op0=ALU.subtract,
                                            op1=ALU.mult)
                    if not ln_g1:
                        nc.vector.tensor_mul(out=xt, in0=xt, in1=G1)
                    if not ln_b0:
                        nc.vector.tensor_add(out=xt, in0=xt, in1=B1t)
                    for d in range(KD):
                        ptr = ps_tr.tile([P, P], FP32, tag="tr")
                        nc.tensor.transpose(ptr, xt[:, d * P:(d + 1) * P],
                                            ident)
                        tf = xnpool.tile([P, P], FP32, tag=f"tmpT{t % 8}_{d}")
                        nc.vector.tensor_copy(tf, ptr)
                        tmpT[(t, d)] = tf
                        nc.vector.tensor_copy(
                            xgT_bf[d][:, t * P:(t + 1) * P], ptr)
                for t in tiles:
                    plg = ps_lg.tile([P, E], FP32, tag="lg")
                    for d in range(KD):
                        nc.tensor.matmul(plg, tmpT[(t, d)], wg_sb[:, d, :],
                                         start=(d == 0), stop=(d == KD - 1))
                    lg = rpool.tile([P, E], FP32, tag="lg_sb")
                    nc.vector.tensor_copy(lg, plg)
                    mx = rpool.tile([P, 8], FP32, tag="mx")
                    nc.vector.max(out=mx, in_=lg)
                    neg_m1 = rpool.tile([P, 1], FP32, tag="neg_m1")
                    nc.vector.tensor_scalar_mul(neg_m1, mx[:, 0:1], -1.0)
                    expl = rpool.tile([P, E], FP32, tag="expl")
                    nc.scalar.activation(out=expl, in_=lg, func=AF.Exp,
                                         bias=neg_m1, scale=1.0)
                    mask = rpool.tile([P, E], FP32, tag="mask")
                    nc.vector.tensor_scalar(out=mask, in0=lg,
                                            scalar1=mx[:, 1:2],
                                            scalar2=None, op0=ALU.is_ge)
                    nc.vector.tensor_mul(out=expl, in0=expl, in1=mask)
                    den = rpool.tile([P, 1], FP32, tag="den")
                    nc.vector.reduce_sum(out=den, in_=expl, axis=AX.X)
                    rec = rpool.tile([P, 1], FP32, tag="rec")
                    nc.vector.reciprocal(out=rec, in_=den)
                    gnum = rpool.tile([P, E], FP32, tag="gnum")
                    nc.vector.tensor_mul(out=gnum, in0=expl, in1=sel_bc)
                    gsum = rpool.tile([P, 1], FP32, tag="gsum")
                    nc.vector.reduce_sum(out=gsum, in_=gnum, axis=AX.X)
                    gt = ppool.tile([P, 1], FP32, tag=f"gt{t}")
                    nc.vector.tensor_scalar(out=gt, in0=gsum, scalar1=rec,
                                            scalar2=None, op0=ALU.mult)
                    gts[t] = gt
                    soh = ppool.tile([P, S], BF16, tag=f"soh{t}")
                    nc.vector.tensor_scalar(out=soh, in0=iota_sb,
                                            scalar1=pos_sb[:, t:t + 1],
                                            scalar2=None, op0=ALU.is_equal)
                    sel_oh[t] = soh

            def emit_batch(b):
                """Position batch b into the paired partial layout; RS when
                the pair is complete."""
                pair, half = b // 2, b % 2
                chunks = _cover(b)
                for mt in range(KD):
                    pp = ps_p.tile([P, D], FP32, tag="pp")
                    for i, (ct, r, n) in enumerate(chunks):
                        nc.tensor.matmul(
                            pp,
                            sel_oh[ct][r:r + n, mt * P:(mt + 1) * P],
                            o_bf[ct][r:r + n, :],
                            start=(i == 0), stop=(i == len(chunks) - 1))
                    part = cpool.tile([P, D], BF16, tag="part")
                    nc.vector.tensor_copy(part, pp)
                    ro = 256 * mt + 64 * half
                    nc.sync.dma_start(
                        out=partial_dram[pair][ro:ro + 64, :],
                        in_=part[0:64, :])
                    nc.sync.dma_start(
                        out=partial_dram[pair][ro + 128:ro + 192, :],
                        in_=part[64:128, :])
                if half == 1:
                    nc.gpsimd.collective_compute(
                        "ReduceScatter", ALU.add, replica_groups=rg,
                        ins=[partial_dram[pair].ap()],
                        outs=[rs_out_dram[pair].ap()])

            def seg_ffn(w):
                """FFN over wave w's slot tiles + positioning/RS as ready."""
                g0 = w * WT
                width = WT * P
                hts = []
                for m in range(KH):
                    ph = ps_h.tile([P, width], FP32, tag="ph")
                    for k in range(KD):
                        nc.tensor.matmul(
                            ph,
                            w1_bf[k][:, m * P:(m + 1) * P],
                            xgT_bf[k][:, g0 * P:g0 * P + width],
                            start=(k == 0), stop=(k == KD - 1))
                    ht = hpool.tile([P, width], BF16, tag=f"ht{m}")
                    nc.scalar.activation(out=ht, in_=ph,
                                         func=AF.Gelu_apprx_tanh,
                                         bias=b1_sb[:, m:m + 1], scale=1.0)
                    hts.append(ht)
                for tt in range(WT):
                    g = g0 + tt
                    po = ps_o.tile([P, D], FP32, tag="po")
                    for m in range(KH):
                        nc.tensor.matmul(po,
                                         hts[m][:, tt * P:(tt + 1) * P],
                                         w2_bf[m],
                                         start=(m == 0), stop=(m == KH - 1))
                    if not b2_0:
                        nc.vector.tensor_add(out=po, in0=po, in1=B2)
                    ob = opool.tile([P, D], BF16, tag=f"o{g}")
                    nc.vector.tensor_scalar_mul(ob, po, gts[g])
                    o_bf[g] = ob
                    for b in _POS_AFTER.get(g, []):
                        emit_batch(b)

            # software-pipelined: wave A runs one wave ahead of the FFN
            wave_A(0)
            wave_A(1)
            seg_ffn(0)
            wave_A(2)
            seg_ffn(1)
            seg_ffn(2)

            # ---------------- residual + final LayerNorm (pairs) ----------------
            # Every pair's chain starts with a multiply-by-zero read of the
            # LAST FFN output tile: a real data dependency that stops the
            # static Tile scheduler (which models collectives as fast) from
            # hoisting these RS-gated ops into the FFN engine streams, where
            # they would stall vector/PE on real hardware.
            ys, mvs = [], []
            for p in range(NPAIR):
                ybf = fpool.tile([P, D], BF16, tag="ybf")
                nc.sync.dma_start(out=ybf, in_=rs_out_dram[p].ap())
                xres = fpool.tile([P, D], FP32, tag="xres")
                nc.sync.dma_start(out=xres, in_=x_res[p, :, :])
                y = fpool.tile([P, D], FP32, tag="y", bufs=NPAIR)
                nc.vector.scalar_tensor_tensor(out=y, in0=o_bf[NTILE - 1],
                                               scalar=0.0, in1=ybf,
                                               op0=ALU.mult, op1=ALU.add)
                nc.vector.tensor_add(out=y, in0=y, in1=xres)
                stats = fpool.tile([P, 6], FP32, tag="fstats")
                nc.vector.bn_stats(out=stats, in_=y)
                mv = fpool.tile([P, 2], FP32, tag="fmv", bufs=NPAIR)
                nc.vector.bn_aggr(out=mv, in_=stats)
                ys.append(y)
                mvs.append(mv)
            for p in range(NPAIR):
                y = ys[p]
                sd = fpool.tile([P, 1], FP32, tag="fsd")
                nc.scalar.activation(out=sd, in_=mvs[p][:, 1:2], func=AF.Sqrt,
                                     bias=eps_t, scale=1.0)
                rstd = fpool.tile([P, 1], FP32, tag="frstd")
                nc.vector.reciprocal(out=rstd, in_=sd)
                nc.vector.tensor_scalar(out=y, in0=y, scalar1=mvs[p][:, 0:1],
                                        scalar2=rstd, op0=ALU.subtract,
                                        op1=ALU.mult)
                if not out_g1:
                    nc.vector.tensor_mul(out=y, in0=y, in1=OG)
                if not out_b0:
                    nc.vector.tensor_add(out=y, in0=y, in1=OB)
                nc.sync.dma_start(out=out_d[p, :, :], in_=y)

    nc.finalize()
    return nc


_NC_CACHE = {}


def _get_nc(flags):
    if flags not in _NC_CACHE:
        _NC_CACHE[flags] = build(*flags)
    return _NC_CACHE[flags]


def _dispatch(xf, Wg, ln_g, ln_b):
    """Host routing decision: which experts each token is sent to."""
    m = xf.mean(1, keepdims=True)
    xc = xf - m
    v = (xc * xc).mean(1, keepdims=True)
    xn = xc / np.sqrt(v + EPS) * ln_g + ln_b
    logits = xn.astype(np.float32) @ Wg.astype(np.float32)   # [T, E]
    srt = np.sort(logits, axis=1)
    # Superset of the device's bf16-router top-2: two bf16-rounded logits
    # can flip a pairwise gap by at most ~2*4.4e-3 on this data; 1.5e-2
    # leaves ~70% headroom on top of that bound.
    thr = srt[:, -2] - 1.5e-2
    disp = logits >= thr[:, None]    # [T, E] bool
    return logits, disp


def kernel(x, Wg, W1, b1, W2, b2, ln_g, ln_b, out_g, out_b, **_run_kwargs):
    x = np.ascontiguousarray(x, dtype=np.float32)
    xf = x.reshape(T, D)
    flags = (bool(np.all(ln_g == 1)), not np.any(ln_b),
             bool(np.all(out_g == 1)), not np.any(out_b), not np.any(b2))
    nc = _get_nc(flags)
    logits, disp = _dispatch(xf, np.asarray(Wg, np.float32),
                             np.asarray(ln_g, np.float32),
                             np.asarray(ln_b, np.float32))
    iota_sb = np.broadcast_to(np.arange(S, dtype=np.float32), (P, S))
    iota_sb = np.ascontiguousarray(iota_sb)
    wg_t = np.asarray(Wg, np.float32).reshape(KD, P, E).transpose(1, 0, 2)
    wg_t = np.ascontiguousarray(wg_t.reshape(P, KD * E))
    b1_t = np.ascontiguousarray(
        np.asarray(b1, np.float32).reshape(-1, KH, P).transpose(0, 2, 1))
    x_res_all = xf.reshape(B, N_CORES, PF, D)  # [batch, core, 64, D]
    W1_bf = np.ascontiguousarray(W1).astype(ml_dtypes.bfloat16)
    W2_bf = np.ascontiguousarray(W2).astype(ml_dtypes.bfloat16)
    in_maps = []
    for i in range(N_CORES):
        sel = np.zeros((E,), dtype=np.float32)
        sel[i] = 1.0
        sel_sb = np.ascontiguousarray(np.broadcast_to(sel, (P, E)))
        xg = np.empty((NSLOT, D), dtype=np.float32)
        pos = np.full((NSLOT,), PAD_POS, dtype=np.float32)
        for b in range(B):
            tok = np.nonzero(disp[b * S:(b + 1) * S, i])[0]
            if len(tok) > CAP:   # overflow: keep the highest-logit tokens
                keep = np.argsort(-logits[b * S + tok, i])[:CAP]
                tok = tok[keep]
            n = len(tok)
            base = b * CAP
            xg[base:base + n] = xf[b * S + tok]
            xg[base + n:base + CAP] = xf[b * S]     # pads: any real row
            pos[base:base + n] = tok.astype(np.float32)
        # pos as [P, NTILE]: pos_sb[p, t] = pos[t*P + p]
        pos_t = np.ascontiguousarray(pos.reshape(NTILE, P).T)
        in_maps.append({
            "xg": xg,
            "pos": pos_t,
            "iota": iota_sb,
            "x_res": np.ascontiguousarray(
                x_res_all[:, i].reshape(NPAIR, 2 * PF, D)),
            "Wgt": wg_t,
            "W1e": W1_bf[i],
            "b1e": b1_t[i],
            "W2e": W2_bf[i],
            "b2e": np.ascontiguousarray(b2[i], dtype=np.float32),
            "ln_g": np.ascontiguousarray(ln_g, dtype=np.float32),
            "ln_b": np.ascontiguousarray(ln_b, dtype=np.float32),
            "out_g": np.ascontiguousarray(out_g, dtype=np.float32),
            "out_b": np.ascontiguousarray(out_b, dtype=np.float32),
            "sel": sel_sb,
        })
    res = run_bass_kernel_spmd(nc, in_maps, list(range(N_CORES)),
                               **_run_kwargs)
    out = np.empty((T, D), dtype=np.float32)
    for i in range(N_CORES):
        oc = res.results[i]["out"]  # [NPAIR, 2*PF, D]
        for p in range(NPAIR):
            out[S * 2 * p + PF * i: S * 2 * p + PF * (i + 1)] = oc[p][:PF]
            out[S * (2 * p + 1) + PF * i:
                S * (2 * p + 1) + PF * (i + 1)] = oc[p][PF:]
    kernel.last_results = res
    return out.reshape(B, S, D)


# revision 3
# speedup vs baseline: 1.0850x; 1.0850x over previous
"""ConfSMoE Trainium2 kernel — capacity-based expert dispatch across 8 cores.

Strategy
-------------
B,S,D,E,H = 8,512,512,8,2048; top-2-of-8 confidence-weighted MoE.
Dense expert compute (baseline) does 4x the needed FLOPs. Instead:

- Host computes the routing DECISION only (which tokens go to which
  expert) and gathers, per core e, the raw x rows of the tokens whose
  top-2 (plus near-ties within 1.5e-2) includes expert e, grouped by
  source batch with a fixed per-(expert,batch) capacity CAP=192
  (observed max cell 148).  Pads point at a real row with
  position=4096 (never selected by the one-hot).  All parameter-style
  inputs arrive pre-laid-out [128, n] so every DMA is contiguous.
- Core e (all math on device): LayerNorm its 1536 slots (fp32 stats),
  PE-transpose, bf16 router logits, softmax + top-2 mask (Max8/is_ge)
  + renormalized confidence of its own expert; FFN in bf16 over
  512-slot segments; gate-scale on vector; weights arrive pre-cast
  bf16 on the gpsimd DMA queue while activations use the sync queue.
- Positioning: one-hot slot->position matrices (device-built from the
  host slot-position vector vs an iota row) matmul the slot-major
  outputs into token-position order; partial rows are written in a
  core-interleaved PAIRED layout so one bf16 ReduceScatter serves two
  batches (4 collectives total, fired as soon as the second batch of
  the pair is positioned, overlapping the remaining FFN segments).
- Each core ends with 64 rows of each batch (two batches stacked per
  RS): residual + final LayerNorm on [128, D] pair tiles.

Engine budget: PE does transposes/router/FFN/positioning; scalar only
sqrt/exp/gelu in contiguous blocks (minimal activation-table reloads);
vector does LN/router vector work and gate scales; gpsimd does weight
DMAs, partial bf16 casts and the collectives.

Device routing is self-consistent across cores; host dispatch only
needs to be a superset of the device's top-2, guaranteed by also
dispatching experts within 1.5e-2 of the 2nd-highest logit (the fp32
router on device matches the host decision far inside that margin).
"""

import numpy as np
import ml_dtypes

import concourse.bass as bass
import concourse.mybir as mybir
import concourse.tile as tile
from concourse import bacc
from concourse.bass_utils import run_bass_kernel_spmd
from concourse.masks import make_identity

B, S, D, E, H = 8, 512, 512, 8, 2048
N_CORES = 8
T = B * S              # 4096 tokens
EPS = 1e-5
P = 128                # partitions
KD = D // P            # 4  D-tiles
KH = H // P            # 16 H-tiles
CAP = 192              # slots per (expert, batch); CAP*b % 128 must be in
                       # {0, 64}: matmul base partition must be 0/32/64
NSLOT = B * CAP        # 1536 slots per core
NTILE = NSLOT // P     # 12 slot tiles
WT = 4                 # slot tiles per wave / FFN segment
NW = NTILE // WT       # 3 waves
PF = S // N_CORES      # 64 output rows per (core, batch)
NPAIR = B // 2         # 4 paired ReduceScatters
PAD_POS = float(T)     # one-hot never matches

FP32 = mybir.dt.float32
BF16 = mybir.dt.bfloat16
AF = mybir.ActivationFunctionType
ALU = mybir.AluOpType
AX = mybir.AxisListType


def _bc(dram_param, p, n):
    """[n]-shaped DRAM tensor broadcast to [p, n] via a step-0 partition dim."""
    a = dram_param.ap()
    return bass.AP(tensor=a.tensor, offset=a.offset, ap=[[0, p]] + list(a.ap))


def _cover(b):
    """Slot-tile chunks (tile, row0, nrows) covering batch b's CAP slots."""
    lo, hi = CAP * b, CAP * (b + 1)
    chunks = []
    while lo < hi:
        t, r = lo // P, lo % P
        n = min(P - r, hi - lo)
        chunks.append((t, r, n))
        lo += n
    return chunks


# batches whose last covering slot tile is g
_POS_AFTER = {}
for _b in range(B):
    _g = (CAP * (_b + 1) + P - 1) // P - 1
    _POS_AFTER.setdefault(_g, []).append(_b)


def build(ln_g1=False, ln_b0=False, out_g1=False, out_b0=False, b2_0=False):
    nc = bacc.Bacc("TRN2", target_bir_lowering=False, debug=False,
                   num_devices=N_CORES)

    # ---------------- I/O (host pre-laid-out, contiguous DMAs) ----------------
    xg_d = nc.declare_dram_parameter("xg", [NSLOT, D], FP32, isOutput=False)
    pos_d = nc.declare_dram_parameter("pos", [P, NTILE], FP32, isOutput=False)
    iota_d = nc.declare_dram_parameter("iota", [P, S], FP32, isOutput=False)
    x_res = nc.declare_dram_parameter("x_res", [NPAIR, 2 * PF, D], FP32,
                                      isOutput=False)
    wg_d = nc.declare_dram_parameter("Wgt", [P, KD * E], FP32, isOutput=False)
    W1_d = nc.declare_dram_parameter("W1e", [D, H], BF16, isOutput=False)
    b1_d = nc.declare_dram_parameter("b1e", [P, KH], FP32, isOutput=False)
    W2_d = nc.declare_dram_parameter("W2e", [H, D], BF16, isOutput=False)
    b2_d = nc.declare_dram_parameter("b2e", [D], FP32, isOutput=False)
    ln_g_d = nc.declare_dram_parameter("ln_g", [D], FP32, isOutput=False)
    ln_b_d = nc.declare_dram_parameter("ln_b", [D], FP32, isOutput=False)
    out_g_d = nc.declare_dram_parameter("out_g", [D], FP32, isOutput=False)
    out_b_d = nc.declare_dram_parameter("out_b", [D], FP32, isOutput=False)
    sel_d = nc.declare_dram_parameter("sel", [P, E], FP32, isOutput=False)
    out_d = nc.declare_dram_parameter("out", [NPAIR, 2 * PF, D], FP32,
                                      isOutput=True)

    # ------------- internal DRAM (paired collectives) -------------
    # partial2[p] rows: 128*j + 64*half + r  ->  core j, batch 2p+half, row r
    partial_dram = [nc.dram_tensor(f"partial{p}", [2 * S, D], BF16)
                    for p in range(NPAIR)]
    rs_out_dram = [nc.dram_tensor(f"rs_out{p}", [2 * PF, D], BF16)
                   for p in range(NPAIR)]

    rg = [list(range(N_CORES))]

    with tile.TileContext(nc) as tc:
        with (
            tc.tile_pool(name="params", bufs=1) as ppool,
            tc.tile_pool(name="wts", bufs=1) as wpool,
            tc.tile_pool(name="xst", bufs=8) as xpool,
            tc.tile_pool(name="xnT", bufs=1) as xnpool,
            tc.tile_pool(name="route", bufs=2) as rpool,
            tc.tile_pool(name="hT", bufs=1) as hpool,
            tc.tile_pool(name="ob", bufs=1) as opool,
            tc.tile_pool(name="part", bufs=4) as cpool,
            tc.tile_pool(name="fin", bufs=2) as fpool,
            tc.tile_pool(name="ps_tr", bufs=1, space="PSUM") as ps_tr,
            tc.tile_pool(name="ps_lg", bufs=1, space="PSUM") as ps_lg,
            tc.tile_pool(name="ps_h", bufs=2, space="PSUM") as ps_h,
            tc.tile_pool(name="ps_o", bufs=2, space="PSUM") as ps_o,
            tc.tile_pool(name="ps_p", bufs=2, space="PSUM") as ps_p,
        ):
            # ---------------- constants / params ----------------
            G1 = B1t = OG = OB = None
            if not ln_g1:
                G1 = ppool.tile([P, D], FP32, tag="G1")
                nc.gpsimd.dma_start(out=G1, in_=_bc(ln_g_d, P, D))
            if not ln_b0:
                B1t = ppool.tile([P, D], FP32, tag="B1t")
                nc.gpsimd.dma_start(out=B1t, in_=_bc(ln_b_d, P, D))
            if not out_g1:
                OG = ppool.tile([P, D], FP32, tag="OG")
                nc.gpsimd.dma_start(out=OG, in_=_bc(out_g_d, P, D))
            if not out_b0:
                OB = ppool.tile([P, D], FP32, tag="OB")
                nc.gpsimd.dma_start(out=OB, in_=_bc(out_b_d, P, D))
            if not b2_0:
                B2 = ppool.tile([P, D], FP32, tag="B2")
                nc.gpsimd.dma_start(out=B2, in_=_bc(b2_d, P, D))

            eps_t = ppool.tile([P, 1], FP32, tag="eps")
            nc.vector.memset(eps_t, EPS)

            ident = ppool.tile([P, P], FP32, tag="ident")
            make_identity(nc, ident)

            # first-wave activations ahead of everything on the sync queue
            xts = [None] * NTILE
            for t in range(WT):
                xt = xpool.tile([P, D], FP32, tag="xg_t")
                nc.sync.dma_start(out=xt, in_=xg_d[t * P:(t + 1) * P, :])
                xts[t] = xt

            wg_sb = ppool.tile([P, KD, E], FP32, tag="wg_sb")
            nc.sync.dma_start(out=wg_sb, in_=wg_d.ap())
            b1_sb = ppool.tile([P, KH], FP32, tag="b1_sb")
            nc.sync.dma_start(out=b1_sb, in_=b1_d.ap())
            pos_sb = ppool.tile([P, NTILE], FP32, tag="pos_sb")
            nc.sync.dma_start(out=pos_sb, in_=pos_d.ap())
            iota_sb = ppool.tile([P, S], FP32, tag="iota_sb")
            nc.sync.dma_start(out=iota_sb, in_=iota_d.ap())
            sel_bc = ppool.tile([P, E], FP32, tag="sel_bc")
            nc.sync.dma_start(out=sel_bc, in_=sel_d.ap())

            # weights: pre-cast bf16, on the gpsimd DMA queue
            w1_bf = []
            for k in range(KD):
                wbf = wpool.tile([P, H], BF16, tag=f"w1_bf{k}")
                nc.gpsimd.dma_start(out=wbf, in_=W1_d[k * P:(k + 1) * P, :])
                w1_bf.append(wbf)
            w2_bf = []
            for m in range(KH):
                wbf = wpool.tile([P, D], BF16, tag=f"w2_bf{m}")
                nc.gpsimd.dma_start(out=wbf, in_=W2_d[m * P:(m + 1) * P, :])
                w2_bf.append(wbf)

            xgT_bf = []        # [KD] tiles [P, NSLOT] bf16 (FFN rhs / router lhsT)
            for d in range(KD):
                xd = xnpool.tile([P, NSLOT], BF16, tag=f"xgT{d}")
                xgT_bf.append(xd)

            gts = [None] * NTILE     # [P, 1] fp32: own-expert confidence
            sel_oh = [None] * NTILE  # [P, S] bf16: slot -> position one-hot
            o_bf = [None] * NTILE    # [P, D] bf16: gate-scaled expert outputs
            tmpT = {}                # (t, d) -> [P, P] fp32 (fp32 router lhsT)

            def wave_A(w):
                """LayerNorm + transpose + router for slot tiles of wave w."""
                tiles = range(w * WT, (w + 1) * WT)
                for t in tiles:
                    if xts[t] is None:
                        xt = xpool.tile([P, D], FP32, tag="xg_t")
                        nc.sync.dma_start(out=xt,
                                          in_=xg_d[t * P:(t + 1) * P, :])
                        xts[t] = xt
                for t in tiles:
                    xt = xts[t]
                    stats = rpool.tile([P, 6], FP32, tag="stats")
                    nc.vector.bn_stats(out=stats, in_=xt)
                    mv = rpool.tile([P, 2], FP32, tag="mv")
                    nc.vector.bn_aggr(out=mv, in_=stats)
                    sd = rpool.tile([P, 1], FP32, tag="sd")
                    nc.scalar.activation(out=sd, in_=mv[:, 1:2], func=AF.Sqrt,
                                         bias=eps_t, scale=1.0)
                    rstd = rpool.tile([P, 1], FP32, tag="rstd")
                    nc.vector.reciprocal(out=rstd, in_=sd)
                    nc.vector.tensor_scalar(out=xt, in0=xt, scalar1=mv[:, 0:1],
                                            scalar2=rstd, op0=ALU.subtract,
                                            op1=ALU.mult)
                    if not ln_g1:
                        nc.vector.tensor_mul(out=xt, in0=xt, in1=G1)
                    if not ln_b0:
                        nc.vector.tensor_add(out=xt, in0=xt, in1=B1t)
                    for d in range(KD):
                        ptr = ps_tr.tile([P, P], FP32, tag="tr")
                        nc.tensor.transpose(ptr, xt[:, d * P:(d + 1) * P],
                                            ident)
                        tf = xnpool.tile([P, P], FP32, tag=f"tmpT{t % 8}_{d}")
                        nc.vector.tensor_copy(tf, ptr)
                        tmpT[(t, d)] = tf
                        nc.vector.tensor_copy(
                            xgT_bf[d][:, t * P:(t + 1) * P], ptr)
                for t in tiles:
                    plg = ps_lg.tile([P, E], FP32, tag="lg")
                    for d in range(KD):
                        nc.tensor.matmul(plg, tmpT[(t, d)], wg_sb[:, d, :],
                                         start=(d == 0), stop=(d == KD - 1))
                    lg = rpool.tile([P, E], FP32, tag="lg_sb")
                    nc.vector.tensor_copy(lg, plg)
                    mx = rpool.tile([P, 8], FP32, tag="mx")
                    nc.vector.max(out=mx, in_=lg)
                    neg_m1 = rpool.tile([P, 1], FP32, tag="neg_m1")
                    nc.vector.tensor_scalar_mul(neg_m1, mx[:, 0:1], -1.0)
                    expl = rpool.tile([P, E], FP32, tag="expl")
                    nc.scalar.activation(out=expl, in_=lg, func=AF.Exp,
                                         bias=neg_m1, scale=1.0)
                    mask = rpool.tile([P, E], FP32, tag="mask")
                    nc.vector.tensor_scalar(out=mask, in0=lg,
                                            scalar1=mx[:, 1:2],
                                            scalar2=None, op0=ALU.is_ge)
                    nc.vector.tensor_mul(out=expl, in0=expl, in1=mask)
                    den = rpool.tile([P, 1], FP32, tag="den")
                    nc.vector.reduce_sum(out=den, in_=expl, axis=AX.X)
                    rec = rpool.tile([P, 1], FP32, tag="rec")
                    nc.vector.reciprocal(out=rec, in_=den)
                    gnum = rpool.tile([P, E], FP32, tag="gnum")
                    nc.vector.tensor_mul(out=gnum, in0=expl, in1=sel_bc)
                    gsum = rpool.tile([P, 1], FP32, tag="gsum")
                    nc.vector.reduce_sum(out=gsum, in_=gnum, axis=AX.X)
                    gt = ppool.tile([P, 1], FP32, tag=f"gt{t}")
                    nc.vector.tensor_scalar(out=gt, in0=gsum, scalar1=rec,
                                            scalar2=None, op0=ALU.mult)
                    gts[t] = gt
                    soh = ppool.tile([P, S], BF16, tag=f"soh{t}")
                    nc.vector.tensor_scalar(out=soh, in0=iota_sb,
                                            scalar1=pos_sb[:, t:t + 1],
                                            scalar2=None, op0=ALU.is_equal)
                    sel_oh[t] = soh

            def emit_batch(b):
                """Position batch b into the paired partial layout; RS when
                the pair is complete."""
                pair, half = b // 2, b % 2
                chunks = _cover(b)
                for mt in range(KD):
                    pp = ps_p.tile([P, D], FP32, tag="pp")
                    for i, (ct, r, n) in enumerate(chunks):
                        nc.tensor.matmul(
                            pp,
                            sel_oh[ct][r:r + n, mt * P:(mt + 1) * P],
                            o_bf[ct][r:r + n, :],
                            start=(i == 0), stop=(i == len(chunks) - 1))
                    part = cpool.tile([P, D], BF16, tag="part")
                    nc.vector.tensor_copy(part, pp)
                    ro = 256 * mt + 64 * half
                    nc.sync.dma_start(
                        out=partial_dram[pair][ro:ro + 64, :],
                        in_=part[0:64, :])
                    nc.sync.dma_start(
                        out=partial_dram[pair][ro + 128:ro + 192, :],
                        in_=part[64:128, :])
                if half == 1:
                    nc.gpsimd.collective_compute(
                        "ReduceScatter", ALU.add, replica_groups=rg,
                        ins=[partial_dram[pair].ap()],
                        outs=[rs_out_dram[pair].ap()])

            def seg_ffn(w):
                """FFN over wave w's slot tiles + positioning/RS as ready."""
                g0 = w * WT
                width = WT * P
                hts = []
                for m in range(KH):
                    ph = ps_h.tile([P, width], FP32, tag="ph")
                    for k in range(KD):
                        nc.tensor.matmul(
                            ph,
                            w1_bf[k][:, m * P:(m + 1) * P],
                            xgT_bf[k][:, g0 * P:g0 * P + width],
                            start=(k == 0), stop=(k == KD - 1))
                    ht = hpool.tile([P, width], BF16, tag=f"ht{m}")
                    nc.scalar.activation(out=ht, in_=ph,
                                         func=AF.Gelu_apprx_tanh,
                                         bias=b1_sb[:, m:m + 1], scale=1.0)
                    hts.append(ht)
                for tt in range(WT):
                    g = g0 + tt
                    po = ps_o.tile([P, D], FP32, tag="po")
                    for m in range(KH):
                        nc.tensor.matmul(po,
                                         hts[m][:, tt * P:(tt + 1) * P],
                                         w2_bf[m],
                                         start=(m == 0), stop=(m == KH - 1))
                    if not b2_0:
                        nc.vector.tensor_add(out=po, in0=po, in1=B2)
                    ob = opool.tile([P, D], BF16, tag=f"o{g}")
                    nc.vector.tensor_scalar_mul(ob, po, gts[g])
                    o_bf[g] = ob
                    for b in _POS_AFTER.get(g, []):
                        emit_batch(b)

            # software-pipelined: wave A runs one wave ahead of the FFN
            wave_A(0)
            wave_A(1)
            seg_ffn(0)
            wave_A(2)
            seg_ffn(1)
            seg_ffn(2)

            # ---------------- residual + final LayerNorm (pairs) ----------------
            # Every pair's chain starts with a multiply-by-zero read of the
            # LAST FFN output tile: a real data dependency that stops the
            # static Tile scheduler (which models collectives as fast) from
            # hoisting these RS-gated ops into the FFN engine streams, where
            # they would stall vector/PE on real hardware.
            ys, mvs = [], []
            for p in range(NPAIR):
                ybf = fpool.tile([P, D], BF16, tag="ybf")
                nc.sync.dma_start(out=ybf, in_=rs_out_dram[p].ap())
                xres = fpool.tile([P, D], FP32, tag="xres")
                nc.sync.dma_start(out=xres, in_=x_res[p, :, :])
                y = fpool.tile([P, D], FP32, tag="y", bufs=NPAIR)
                nc.vector.scalar_tensor_tensor(out=y, in0=o_bf[NTILE - 1],
                                               scalar=0.0, in1=ybf,
                                               op0=ALU.mult, op1=ALU.add)
                nc.vector.tensor_add(out=y, in0=y, in1=xres)
                stats = fpool.tile([P, 6], FP32, tag="fstats")
                nc.vector.bn_stats(out=stats, in_=y)
                mv = fpool.tile([P, 2], FP32, tag="fmv", bufs=NPAIR)
                nc.vector.bn_aggr(out=mv, in_=stats)
                ys.append(y)
                mvs.append(mv)
            for p in range(NPAIR):
                y = ys[p]
                sd = fpool.tile([P, 1], FP32, tag="fsd")
                nc.scalar.activation(out=sd, in_=mvs[p][:, 1:2], func=AF.Sqrt,
                                     bias=eps_t, scale=1.0)
                rstd = fpool.tile([P, 1], FP32, tag="frstd")
                nc.vector.reciprocal(out=rstd, in_=sd)
                nc.vector.tensor_scalar(out=y, in0=y, scalar1=mvs[p][:, 0:1],
                                        scalar2=rstd, op0=ALU.subtract,
                                        op1=ALU.mult)
                if not out_g1:
                    nc.vector.tensor_mul(out=y, in0=y, in1=OG)
                if not out_b0:
                    nc.vector.tensor_add(out=y, in0=y, in1=OB)
                nc.sync.dma_start(out=out_d[p, :, :], in_=y)

    nc.finalize()
    return nc


_NC_CACHE = {}


def _get_nc(flags):
    if flags not in _NC_CACHE:
        _NC_CACHE[flags] = build(*flags)
    return _NC_CACHE[flags]


def _dispatch(xf, Wg, ln_g, ln_b):
    """Host routing decision: which experts each token is sent to."""
    m = xf.mean(1, keepdims=True)
    xc = xf - m
    v = (xc * xc).mean(1, keepdims=True)
    xn = xc / np.sqrt(v + EPS) * ln_g + ln_b
    logits = xn.astype(np.float32) @ Wg.astype(np.float32)   # [T, E]
    srt = np.sort(logits, axis=1)
    # Superset of the device's bf16-router top-2: two bf16-rounded logits
    # can flip a pairwise gap by at most ~2*4.4e-3 on this data; 1.5e-2
    # leaves ~70% headroom on top of that bound.
    thr = srt[:, -2] - 1.5e-2
    disp = logits >= thr[:, None]    # [T, E] bool
    return logits, disp


def kernel(x, Wg, W1, b1, W2, b2, ln_g, ln_b, out_g, out_b, **_run_kwargs):
    x = np.ascontiguousarray(x, dtype=np.float32)
    xf = x.reshape(T, D)
    flags = (bool(np.all(ln_g == 1)), not np.any(ln_b),
             bool(np.all(out_g == 1)), not np.any(out_b), not np.any(b2))
    nc = _get_nc(flags)
    logits, disp = _dispatch(xf, np.asarray(Wg, np.float32),
                             np.asarray(ln_g, np.float32),
                             np.asarray(ln_b, np.float32))
    iota_sb = np.broadcast_to(np.arange(S, dtype=np.float32), (P, S))
    iota_sb = np.ascontiguousarray(iota_sb)
    wg_t = np.asarray(Wg, np.float32).reshape(KD, P, E).transpose(1, 0, 2)
    wg_t = np.ascontiguousarray(wg_t.reshape(P, KD * E))
    b1_t = np.ascontiguousarray(
        np.asarray(b1, np.float32).reshape(-1, KH, P).transpose(0, 2, 1))
    x_res_all = xf.reshape(B, N_CORES, PF, D)  # [batch, core, 64, D]
    W1_bf = np.ascontiguousarray(W1).astype(ml_dtypes.bfloat16)
    W2_bf = np.ascontiguousarray(W2).astype(ml_dtypes.bfloat16)
    in_maps = []
    for i in range(N_CORES):
        sel = np.zeros((E,), dtype=np.float32)
        sel[i] = 1.0
        sel_sb = np.ascontiguousarray(np.broadcast_to(sel, (P, E)))
        xg = np.empty((NSLOT, D), dtype=np.float32)
        pos = np.full((NSLOT,), PAD_POS, dtype=np.float32)
        for b in range(B):
            tok = np.nonzero(disp[b * S:(b + 1) * S, i])[0]
            if len(tok) > CAP:   # overflow: keep the highest-logit tokens
                keep = np.argsort(-logits[b * S + tok, i])[:CAP]
                tok = tok[keep]
            n = len(tok)
            base = b * CAP
            xg[base:base + n] = xf[b * S + tok]
            xg[base + n:base + CAP] = xf[b * S]     # pads: any real row
            pos[base:base + n] = tok.astype(np.float32)
        # pos as [P, NTILE]: pos_sb[p, t] = pos[t*P + p]
        pos_t = np.ascontiguousarray(pos.reshape(NTILE, P).T)
        in_maps.append({
            "xg": xg,
            "pos": pos_t,
            "iota": iota_sb,
            "x_res": np.ascontiguousarray(
                x_res_all[:, i].reshape(NPAIR, 2 * PF, D)),
            "Wgt": wg_t,
            "W1e": W1_bf[i],
            "b1e": b1_t[i],
            "W2e": W2_bf[i],
            "b2e": np.ascontiguousarray(b2[i], dtype=np.float32),
            "ln_g": np.ascontiguousarray(ln_g, dtype=np.float32),
            "ln_b": np.ascontiguousarray(ln_b, dtype=np.float32),
            "out_g": np.ascontiguousarray(out_g, dtype=np.float32),
            "out_b": np.ascontiguousarray(out_b, dtype=np.float32),
            "sel": sel_sb,
        })
    res = run_bass_kernel_spmd(nc, in_maps, list(range(N_CORES)),
                               **_run_kwargs)
    out = np.empty((T, D), dtype=np.float32)
    for i in range(N_CORES):
        oc = res.results[i]["out"]  # [NPAIR, 2*PF, D]
        for p in range(NPAIR):
            out[S * 2 * p + PF * i: S * 2 * p + PF * (i + 1)] = oc[p][:PF]
            out[S * (2 * p + 1) + PF * i:
                S * (2 * p + 1) + PF * (i + 1)] = oc[p][PF:]
    kernel.last_results = res
    return out.reshape(B, S, D)
